# revision 1
# baseline (speedup 1.0000x reference)
"""BitConv2d forward on 8 Trainium2 NeuronCores (SPMD data-parallel).

Strategy:
  - Shard batch (32) -> 4 images per core; replicate the tiny bit-plane
    weights/scales on every core. No collectives needed (forward only).
  - On each core, reconstruct the integer weight planes on device:
        W_int[o,i,kh,kw] = sum_b (pweight-nweight)[...,b] * 2^(3-b)   (exact, in [-15,15])
    and fold scale/15 plus the bias into the PSUM->SBUF epilogue.
  - 3x3 same-pad conv as 9 accumulating matmuls per output tile. The image is
    stored zero-padded (114x114) and row-flattened in SBUF twice: partitions
    0:64 hold padded rows 0..57 (top half), partitions 64:128 hold padded rows
    56..113 (bottom half). Each tap's stationary operand is the block-diagonal
    [[W_t, 0], [0, W_t]] (128x128), so one matmul produces the tap contribution
    for TWO output positions per streamed column (full 128-wide M).
  - Matmuls run in float32r (tf32-like: full-rate at N>=256, ~11-bit mantissa).
  - Epilogue (scale+bias) on DVE via tensor_scalar; 6 PSUM banks in flight;
    all pools share one scope so weight-prep overlaps the first image load.
"""

import numpy as np

B, C, H, W = 32, 64, 112, 112
NB = 4
CORES = 8
BPC = B // CORES  # images per core

WP = H + 2  # padded width/height = 114
HALF = H // 2  # 56 output rows per position-group
XC_DATA = 58 * WP  # 6612 data columns per partition block
XC = 59 * WP  # + one zero row (junk-column tap reads run past the data)
OUTC = HALF * WP  # 6384 output columns per group

# N-tiles: all >=256 so float32r runs at full rate (PSUM bank caps at 512 fp32)
N_TILES = [(i * 512, 512) for i in range(11)] + [(5632, 376), (6008, 376)]
TAP_OFFS = [kh * WP + kw for kh in range(3) for kw in range(3)]

_CACHE = {}


def _build():
    if "nc" in _CACHE:
        return _CACHE["nc"]
    import concourse.bacc as bacc
    import concourse.mybir as mybir
    from concourse import tile
    from concourse.masks import make_identity

    f32 = mybir.dt.float32
    f32r = mybir.dt.float32r
    u32 = mybir.dt.uint32
    mult = mybir.AluOpType.mult
    add = mybir.AluOpType.add

    nc = bacc.Bacc("TRN2", target_bir_lowering=False, debug=False, num_devices=CORES)

    x_d = nc.dram_tensor("x", [BPC, C, H, W], f32, kind="ExternalInput").ap()
    pw_d = nc.dram_tensor("pweight", [C, C, 3, 3, NB], f32, kind="ExternalInput").ap()
    nw_d = nc.dram_tensor("nweight", [C, C, 3, 3, NB], f32, kind="ExternalInput").ap()
    sc_d = nc.dram_tensor("scale", [1], f32, kind="ExternalInput").ap()
    pb_d = nc.dram_tensor("pbias", [C, NB], f32, kind="ExternalInput").ap()
    nb_d = nc.dram_tensor("nbias", [C, NB], f32, kind="ExternalInput").ap()
    bs_d = nc.dram_tensor("biasscale", [1], f32, kind="ExternalInput").ap()
    y_d = nc.dram_tensor("y", [BPC, C, H, W], f32, kind="ExternalOutput").ap()

    with tile.TileContext(nc) as tc:
        with (
            tc.tile_pool(name="consts", bufs=1) as consts,
            tc.tile_pool(name="xpool", bufs=2) as xpool,
            tc.tile_pool(name="opool", bufs=2) as opool,
            tc.tile_pool(name="pspool", bufs=7, space="PSUM") as pspool,
            tc.tile_pool(name="psum_t", bufs=1, space="PSUM") as psum_t,
        ):
            ident = consts.tile([C, C], f32, tag="ident")
            make_identity(nc, ident[:])
            lhsT = [
                consts.tile([128, 128], f32r, tag=f"lhsT{t}", name=f"lhsT{t}")
                for t in range(9)
            ]
            scale_vec = consts.tile([128, 1], f32, tag="scale_vec")
            bias_vec = consts.tile([128, 1], f32, tag="bias_vec")

            # ---- weight/bias reconstruction (tiny, runs once; overlaps image-0 DMA) ----
            wp = consts.tile([C, C * 9 * NB], f32, tag="wp")
            wn = consts.tile([C, C * 9 * NB], f32, tag="wn")
            nc.sync.dma_start(wp[:], pw_d.rearrange("o i kh kw b -> o (i kh kw b)"))
            nc.sync.dma_start(wn[:], nw_d.rearrange("o i kh kw b -> o (i kh kw b)"))
            nc.vector.tensor_sub(wp[:], wp[:], wn[:])  # d = p - n
            # bit-combine into tap-major W_int [o, (t, i)]:
            # w = ((d0*8 + d3) + d1*4) + d2*2 via scalar_tensor_tensor chains
            wi = consts.tile([C, 9 * C], f32, tag="wi")
            wt2 = consts.tile([C, 9 * C], f32, tag="wt2")
            wi_v = wi[:].rearrange("p (t i) -> p t i", t=9)
            wt2_v = wt2[:].rearrange("p (t i) -> p t i", t=9)
            d_v = wp[:].rearrange("p (i t b) -> p t i b", t=9, b=NB)
            nc.vector.scalar_tensor_tensor(
                out=wt2_v, in0=d_v[:, :, :, 0], scalar=8.0, in1=d_v[:, :, :, 3],
                op0=mult, op1=add,
            )
            nc.vector.scalar_tensor_tensor(
                out=wi_v, in0=d_v[:, :, :, 1], scalar=4.0, in1=wt2_v,
                op0=mult, op1=add,
            )
            nc.vector.scalar_tensor_tensor(
                out=wt2_v, in0=d_v[:, :, :, 2], scalar=2.0, in1=wi_v,
                op0=mult, op1=add,
            )
            # per-tap block-diagonal lhsT
            for t in range(9):
                wtmp = consts.tile([C, 128], f32, tag=f"wtmp{t % 2}", name=f"wtmp{t}")
                nc.scalar.copy(wtmp[:, 0:C], wt2_v[:, t, :])
                nc.scalar.copy(wtmp[:, C:128], wt2_v[:, t, :])
                ps = psum_t.tile([128, C], f32, tag="tps", name=f"tps{t}")
                nc.tensor.transpose(ps[:], wtmp[:], ident[:])
                nc.gpsimd.memset(lhsT[t][:].bitcast(u32), 0)
                nc.scalar.copy(lhsT[t][0:C, 0:C], ps[0:C, :])
                nc.scalar.copy(lhsT[t][C:128, C:128], ps[C:128, :])
            # bias vector, duplicated across both partition blocks
            pbt = consts.tile([128, NB], f32, tag="pbt")
            nbt = consts.tile([128, NB], f32, tag="nbt")
            nc.sync.dma_start(pbt[0:C, :], pb_d)
            nc.sync.dma_start(pbt[C:128, :], pb_d)
            nc.sync.dma_start(nbt[0:C, :], nb_d)
            nc.sync.dma_start(nbt[C:128, :], nb_d)
            nc.vector.tensor_sub(pbt[:], pbt[:], nbt[:])
            btmp = consts.tile([128, 1], f32, tag="btmp")
            nc.vector.scalar_tensor_tensor(
                out=btmp[:], in0=pbt[:, 0:1], scalar=8.0, in1=pbt[:, 3:4],
                op0=mult, op1=add,
            )
            nc.vector.scalar_tensor_tensor(
                out=bias_vec[:], in0=pbt[:, 1:2], scalar=4.0, in1=btmp[:],
                op0=mult, op1=add,
            )
            nc.vector.scalar_tensor_tensor(
                out=btmp[:], in0=pbt[:, 2:3], scalar=2.0, in1=bias_vec[:],
                op0=mult, op1=add,
            )
            bsv = consts.tile([128, 1], f32, tag="bsv")
            nc.sync.dma_start(bsv[:], bs_d.to_broadcast((128, 1)))
            nc.vector.tensor_mul(btmp[:], btmp[:], bsv[:])
            nc.scalar.mul(bias_vec[:], btmp[:], 1.0 / 15.0)
            nc.sync.dma_start(scale_vec[:], sc_d.to_broadcast((128, 1)))
            nc.scalar.mul(scale_vec[:], scale_vec[:], 1.0 / 15.0)


            # ---- image load pipeline ----
            def load_image(b):
                """Zero the pad strips, then gpsimd cast-DMAs (f32 -> f32r)
                straight into the matmul operand tile. bufs=3 lets loads run
                two images ahead of compute."""
                xs = xpool.tile([128, XC], f32r, tag="xs", name=f"xs{b}", bufs=3)
                v0 = xs[0:C, 0:XC_DATA].rearrange("p (r w) -> p r w", w=WP)
                v1 = xs[C:128, 0:XC_DATA].rearrange("p (r w) -> p r w", w=WP)
                nc.gpsimd.memset(xs[0:C, 0:WP].bitcast(u32), 0)
                nc.gpsimd.memset(v0[:, :, 113:114].bitcast(u32), 0)
                nc.gpsimd.memset(v0[:, 1:58, 0:1].bitcast(u32), 0)
                nc.gpsimd.memset(xs[C:128, 57 * WP : XC_DATA].bitcast(u32), 0)
                nc.gpsimd.memset(v1[:, 0:57, 113:114].bitcast(u32), 0)
                nc.gpsimd.memset(v1[:, 1:57, 0:1].bitcast(u32), 0)
                nc.gpsimd.memset(xs[C:128, 0:1].bitcast(u32), 0)
                nc.gpsimd.memset(xs[:, XC_DATA:XC].bitcast(u32), 0)
                # image rows: block0 = padded rows 0..57, block1 = 56..113
                nc.gpsimd.dma_start(v0[:, 1:58, 1:113], x_d[b, :, 0:57, :])
                nc.gpsimd.dma_start(v1[:, 0:57, 1:113], x_d[b, :, 55:112, :])
                return xs

            xs_next = load_image(0)
            xs_next2 = load_image(1)

            # ---- main conv loop ----
            for b in range(BPC):
                xs = xs_next
                xs_next = xs_next2
                xs_next2 = load_image(b + 2) if b + 2 < BPC else None

                outb = opool.tile([128, OUTC], f32, tag="outb")
                for n0, nt in N_TILES:
                    ps = pspool.tile([128, 512], f32, tag="ps")
                    for t, off in enumerate(TAP_OFFS):
                        nc.tensor.matmul(
                            ps[:, 0:nt],
                            lhsT[t][:],
                            xs[:, n0 + off : n0 + off + nt],
                            start=(t == 0),
                            stop=(t == 8),
                        )
                    nc.scalar.activation(
                        outb[:, n0 : n0 + nt],
                        ps[:, 0:nt],
                        mybir.ActivationFunctionType.Identity,
                        bias=bias_vec[:],
                        scale=scale_vec[:],
                    )
                # stream results out in row chunks so the final DMA is small
                ov = outb[:].rearrange("p (r w) -> p r w", w=WP)
                for r0 in range(0, HALF, 14):
                    nc.sync.dma_start(
                        y_d[b, :, r0 : r0 + 14, :], ov[0:C, r0 : r0 + 14, 0:112]
                    )
                    nc.sync.dma_start(
                        y_d[b, :, HALF + r0 : HALF + r0 + 14, :],
                        ov[C:128, r0 : r0 + 14, 0:112],
                    )

    nc.compile()
    _CACHE["nc"] = nc
    return nc


def _run(inputs, trace=False):
    from concourse.bass_utils import run_bass_kernel_spmd

    nc = _build()
    x = np.ascontiguousarray(np.asarray(inputs["x"], dtype=np.float32))
    shared = {
        "pweight": np.ascontiguousarray(np.asarray(inputs["pweight"], np.float32)),
        "nweight": np.ascontiguousarray(np.asarray(inputs["nweight"], np.float32)),
        "scale": np.ascontiguousarray(np.asarray(inputs["scale"], np.float32)),
        "pbias": np.ascontiguousarray(np.asarray(inputs["pbias"], np.float32)),
        "nbias": np.ascontiguousarray(np.asarray(inputs["nbias"], np.float32)),
        "biasscale": np.ascontiguousarray(np.asarray(inputs["biasscale"], np.float32)),
    }
    in_maps = [dict(shared, x=x[c * BPC : (c + 1) * BPC]) for c in range(CORES)]
    last_err = None
    for attempt in range(3):
        try:
            res = run_bass_kernel_spmd(
                nc, in_maps, core_ids=list(range(CORES)), trace=trace
            )
            out = np.concatenate(
                [res.results[c]["y"] for c in range(CORES)], axis=0
            )
            return out, res.exec_time_ns
        except Exception as e:  # transient NRT_EXEC_UNIT_UNRECOVERABLE recovers on retry
            last_err = e
            import time

            time.sleep(10)
    raise last_err


def kernel(**inputs) -> np.ndarray:
    out, _ = _run(inputs)
    return out

